# revision 14
# baseline (speedup 1.0000x reference)
"""Trainium2 Bass kernel for nn_Encoder_51900384804901.

6-layer post-norm TransformerEncoder (E=1024, NH=16, DFF=4096, relu FFN)
where every token attends only to the first num_ctx=1024 context tokens.

Sharding: data-parallel over batch. B=8 -> one batch element per NeuronCore,
no collectives. Each core runs the full 6-layer encoder on its [2048, 1024]
slice with activations resident in SBUF (feature-major x^T [E, T]) and
weights streamed from HBM.

All-bf16 datapath (PSUM accumulation stays fp32); the PE is kept dense so the
HAM clock gate stays at 2.4 GHz:
  - Softmax denominator reciprocal via reciprocal_approx_fast (single DVE op,
    ~5x faster than iterative divide) so the PSUM po tile frees quickly and
    the PE never stalls on attention-normalize chains. Custom-DVE ops only
    work at base partition 0, so the denominator row (psum row 64) is staged
    through a partition-0 tile first.
  - SBUF pools hoisted out of the layer loop; K/V/FFN weights stream through
    one shared pool so next-layer weight DMAs prefetch during the current
    layer's compute (removes the 22us layer-boundary PE gap).
  - LN rstd via a single Abs_reciprocal_sqrt activation: its act-table set
    also holds square/relu/identity, so the ACT table switches only twice per
    layer (exp set <-> abs_rsqrt set); using ln+exp instead thrashed the
    table ~16x per layer (~124us).
"""

import os
import numpy as np
import ml_dtypes

import concourse.bacc as bacc
import concourse.tile as tile
from concourse import mybir
from concourse import bass_utils

# Problem dims (hardcoded per contract)
L, E, NH, DFF = 6, 1024, 16, 4096
B, S, NC = 8, 2048, 1024
DH = E // NH  # 64
LN_EPS = 1e-5

F32 = mybir.dt.float32
BF16 = mybir.dt.bfloat16

P = 128          # partitions
NQ = S // 512    # 4 q-chunks of 512
NCC = NC // 512  # 2 ctx chunks of 512
ET = E // P      # 8 e-tiles
FT = DFF // P    # 32 f-tiles
KT = NC // P     # 8 ctx k-tiles


def build_encoder():
    nc = bacc.Bacc("TRN2", debug=False)

    xT = nc.dram_tensor("xT", [E, S], BF16, kind="ExternalInput").ap()
    wqkvT = nc.dram_tensor("wqkvT", [L, E, 3 * E], BF16, kind="ExternalInput").ap()
    woT = nc.dram_tensor("woT", [L, E, E], BF16, kind="ExternalInput").ap()
    w1T = nc.dram_tensor("w1T", [L, E, DFF], BF16, kind="ExternalInput").ap()
    w2T = nc.dram_tensor("w2T", [L, DFF, E], BF16, kind="ExternalInput").ap()
    # params[l]: [128, 104] per-partition param columns:
    # 0-7 bq | 8-15 bk | 16-23 bv | 24-31 bo | 32-63 b1 | 64-71 b2
    # 72-79 g1 | 80-87 be1 | 88-95 g2 | 96-103 be2   (col m <-> e-tile m)
    params = nc.dram_tensor("params", [L, P, 104], F32, kind="ExternalInput").ap()
    bvrow = nc.dram_tensor("bvrow", [L, E], F32, kind="ExternalInput").ap()
    onesb = nc.dram_tensor("onesb", [P, 16], BF16, kind="ExternalInput").ap()
    outT = nc.dram_tensor("outT", [E, S], BF16, kind="ExternalOutput").ap()
    debug = os.environ.get("ENC_DEBUG", "") == "1"
    if debug:
        dbg_q = nc.dram_tensor("dbg_q", [E, 512], BF16, kind="ExternalOutput").ap()
        dbg_k = nc.dram_tensor("dbg_k", [E, NC], BF16, kind="ExternalOutput").ap()
        dbg_v = nc.dram_tensor("dbg_v", [KT, P, NH * (DH + 1)], BF16, kind="ExternalOutput").ap()
        dbg_es = nc.dram_tensor("dbg_es", [4, P, 1024], BF16, kind="ExternalOutput").ap()
        dbg_at = nc.dram_tensor("dbg_at", [E, 512], BF16, kind="ExternalOutput").ap()

    AF = mybir.ActivationFunctionType
    OP = mybir.AluOpType

    with tile.TileContext(nc) as tc:
        with tc.tile_pool(name="persist", bufs=1) as pp, \
             tc.tile_pool(name="wstream", bufs=4) as wsp, \
             tc.tile_pool(name="wv", bufs=1) as wvp, \
             tc.tile_pool(name="kv", bufs=1) as kvp, \
             tc.tile_pool(name="att", bufs=1) as att, \
             tc.tile_pool(name="es", bufs=10) as esp, \
             tc.tile_pool(name="sc", bufs=3) as scp, \
             tc.tile_pool(name="hp", bufs=1) as hpool, \
             tc.tile_pool(name="sc2", bufs=3) as sc2, \
             tc.tile_pool(name="parp", bufs=2) as parp:
            # Residual stream x^T, resident for the whole kernel (bf16)
            xt = [pp.tile([P, S], BF16, tag=f"x{i}", name=f"x{i}")
                  for i in range(ET)]
            # ctx columns first (512-col chunks) so layer-0 K/V projection
            # starts as soon as the first 1MB lands
            for c0 in range(0, NC, 512):
                for i in range(ET):
                    nc.sync.dma_start(xt[i][:, c0:c0 + 512],
                                      xT[P * i:P * (i + 1), c0:c0 + 512])
            for i in range(ET):
                nc.sync.dma_start(xt[i][:, NC:S], xT[P * i:P * (i + 1), NC:S])
            ones = pp.tile([P, 1], BF16, name="ones")
            nc.sync.dma_start(ones[:], onesb[:, 0:1])
            eps_t = pp.tile([P, 1], F32, tag="eps", name="eps")
            nc.vector.memset(eps_t[:], LN_EPS)

            def gemm_512(wdram, col_off, mg, rhs_tiles, nk, evict, ps4):
                """out[m] = sum_k W[k, col_off+mg*512+mi*128 ...].T @ rhs[k]
                for the 4 m-subtiles of one 512-wide weight column group.
                ps4: list of 4 one-bank psum APs."""
                c0 = col_off + mg * 512
                for k in range(nk):
                    w = wsp.tile([P, 512], BF16, tag="w", name="w")
                    nc.sync.dma_start(
                        w[:], wdram[P * k:P * (k + 1), c0:c0 + 512])
                    for mi in range(4):
                        nc.tensor.matmul(
                            ps4[mi],
                            w[:, P * mi:P * (mi + 1)],
                            rhs_tiles[k],
                            start=(k == 0), stop=(k == nk - 1))
                for mi in range(4):
                    evict(mg * 4 + mi, ps4[mi])

            def ln_stats(qc, psp, ln_pool):
                """Partition-axis sums + broadcast-form mean / rstd tiles.
                Returns (mean_b, rstd_b) [128,512] f32 SBUF tiles."""
                cs = slice(qc * 512, (qc + 1) * 512)
                s1 = psp.tile([P, 512], F32, tag="ps", name="ps")
                for k in range(ET):
                    nc.tensor.matmul(s1[0:1, :], ones[:], xt[k][:, cs],
                                     start=(k == 0), stop=(k == ET - 1))
                s2 = psp.tile([P, 512], F32, tag="ps", name="ps")
                for k in range(ET):
                    sq = ln_pool.tile([P, 512], BF16, tag="sq", name="sq",
                                      bufs=2)
                    nc.vector.tensor_tensor(sq[:], xt[k][:, cs],
                                            xt[k][:, cs], OP.mult)
                    nc.tensor.matmul(s2[0:1, :], ones[:], sq[:],
                                     start=(k == 0), stop=(k == ET - 1))
                m1 = ln_pool.tile([1, 512], F32, tag="m1", name="m1", bufs=1)
                nc.vector.tensor_scalar_mul(m1[:], s1[0:1, :], 1.0 / E)
                m2 = ln_pool.tile([1, 512], F32, tag="m2", name="m2", bufs=1)
                nc.vector.tensor_scalar_mul(m2[:], s2[0:1, :], 1.0 / E)
                mb = ln_pool.tile([P, 512], F32, tag="mb", name="mb", bufs=4)
                nc.gpsimd.partition_broadcast(mb[:], m1[:])
                vb = ln_pool.tile([P, 512], F32, tag="vb", name="vb", bufs=4)
                nc.gpsimd.partition_broadcast(vb[:], m2[:])
                # var = E[x^2] - mean^2 ;  rstd = 1/sqrt(var+eps).
                # Single ACT op; its table set also holds square/relu/identity
                # so the act table switches only twice per layer.
                msq = ln_pool.tile([P, 512], F32, tag="msq", name="msq",
                                   bufs=1)
                nc.vector.tensor_tensor(msq[:], mb[:], mb[:], OP.mult)
                nc.vector.tensor_tensor(vb[:], vb[:], msq[:], OP.subtract)
                nc.scalar.activation(vb[:], vb[:], AF.Abs_reciprocal_sqrt,
                                     bias=eps_t[:])
                return mb, vb

            def ln_apply(qc, mb, vb, g_col, b_col, par, ln_pool):
                cs = slice(qc * 512, (qc + 1) * 512)
                for k in range(ET):
                    t1 = ln_pool.tile([P, 512], BF16, tag="t1", name="t1",
                                      bufs=2)
                    nc.vector.tensor_tensor(t1[:], xt[k][:, cs], mb[:],
                                            OP.subtract)
                    nc.vector.tensor_tensor(t1[:], t1[:], vb[:], OP.mult)
                    nc.vector.tensor_scalar(
                        xt[k][:, cs], t1[:],
                        par[:, g_col + k:g_col + k + 1],
                        par[:, b_col + k:b_col + k + 1],
                        OP.mult, OP.add)

            nlayers = int(os.environ.get("ENC_NLAYERS", L))
            skip_ffn = os.environ.get("ENC_SKIP_FFN", "") == "1"
            for l in range(nlayers):
                par = parp.tile([P, 104], F32, tag="par", name="par")
                nc.sync.dma_start(par[:], params[l])

                # ---- Phase 1+2: K/V then attention over all q-chunks ----
                kt = [kvp.tile([P, NC], BF16, tag=f"k{i}", name=f"k{i}")
                      for i in range(ET)]
                vp = [kvp.tile([P, NH * (DH + 1)], BF16, tag=f"v{i}",
                               name=f"v{i}") for i in range(KT)]
                with tc.tile_pool(name="psa", bufs=1, space="PSUM") as psa:
                    def s2_tile():
                        return psa.tile([P, 1024], F32, tag="s2",
                                        name="s2", bufs=3)

                    def po_tile():
                        return psa.tile([P, 512], F32, tag="po",
                                        name="po", bufs=2)

                    def gemm_ps4():
                        a, b = s2_tile(), s2_tile()
                        return [a[:, 0:512], a[:, 512:1024],
                                b[:, 0:512], b[:, 512:1024]]

                    bvb = scp.tile([P, E], F32, tag="bvb", name="bvb",
                                   bufs=1)
                    bvr = scp.tile([1, E], F32, tag="bvr", name="bvr",
                                   bufs=1)
                    nc.sync.dma_start(bvr[:], bvrow[l][None, :])
                    nc.gpsimd.partition_broadcast(bvb[:], bvr[:])

                    # K projection (feature-major out), streamed weights
                    for cc in range(NCC):
                        ccs = slice(cc * 512, (cc + 1) * 512)
                        rhs_ctx = [xt[k][:, ccs] for k in range(ET)]

                        def ev_k(m, ps, _ccs=ccs):
                            nc.vector.tensor_scalar_add(
                                kt[m][:, _ccs], ps, par[:, 8 + m:9 + m])
                        for mg in range(2):
                            gemm_512(wqkvT[l], E, mg, rhs_ctx, ET, ev_k,
                                     gemm_ps4())

                    # V projection (token-major out), x_ctx as lhsT
                    wv = [wvp.tile([P, E], BF16, tag=f"wv{k}",
                                   name=f"wv{k}") for k in range(ET)]
                    for k in range(ET):
                        nc.sync.dma_start(
                            wv[k][:],
                            wqkvT[l, P * k:P * (k + 1), 2 * E:3 * E])
                    for t in range(KT):
                        for ch in range(2):
                            ps = po_tile()
                            for k in range(ET):
                                nc.tensor.matmul(
                                    ps[:],
                                    xt[k][:, P * t:P * (t + 1)],
                                    wv[k][:, ch * 512:(ch + 1) * 512],
                                    start=(k == 0),
                                    stop=(k == ET - 1))
                            for hh in range(8):
                                h = ch * 8 + hh
                                nc.vector.tensor_tensor(
                                    vp[t][:, h * 65:h * 65 + 64],
                                    ps[:, hh * 64:(hh + 1) * 64],
                                    bvb[:, h * 64:(h + 1) * 64],
                                    OP.add)
                        ones_cols = vp[t].rearrange(
                            "p (h c) -> p h c", c=DH + 1)[:, :, 64:65]
                        nc.sync.dma_start(ones_cols, onesb[:, :, None])

                    # Attention + out-projection per q-chunk
                    for qc in range(NQ):
                        cs = slice(qc * 512, (qc + 1) * 512)
                        qt = [att.tile([P, 512], BF16, tag=f"q{i}",
                                       name=f"q{i}")
                              for i in range(ET)]
                        rhs_x = [xt[k][:, cs] for k in range(ET)]

                        def ev_q(m, ps):
                            nc.vector.tensor_scalar_add(
                                qt[m][:], ps, par[:, m:m + 1])
                        for mg in range(2):
                            gemm_512(wqkvT[l], 0, mg, rhs_x, ET, ev_q,
                                     gemm_ps4())

                        # attention per head pair (row-tiled quadrants)
                        at = [att.tile([P, 512], BF16, tag=f"a{i}",
                                       name=f"a{i}")
                              for i in range(ET)]
                        for hp_ in range(NH // 2):
                            h0, h1 = 2 * hp_, 2 * hp_ + 1
                            es_pair = [[], []]  # per head: 4 es tiles
                            for kk in range(4):
                                pss = [s2_tile(), s2_tile()]
                                for j in range(2):
                                    ktile = 2 * kk + j
                                    for hi, off in ((0, 0), (1, 64)):
                                        nc.tensor.matmul(
                                            pss[hi][:, j * 512:(j + 1) * 512],
                                            kt[hp_][off:off + 64,
                                                    P * ktile:P * (ktile + 1)],
                                            qt[hp_][off:off + 64, :],
                                            start=True, stop=True)
                                for hi in range(2):
                                    es = esp.tile([P, 1024], BF16,
                                                  tag="es", name="es")
                                    nc.scalar.activation(
                                        es[:], pss[hi][:], AF.Exp,
                                        scale=float(1.0 / np.sqrt(DH)))
                                    if debug and l == 0 and qc == 0 \
                                            and hp_ == 0 and hi == 0:
                                        nc.sync.dma_start(dbg_es[kk], es[:])
                                    es_pair[hi].append(es)
                            for hi, h in ((0, h0), (1, h1)):
                                po = po_tile()
                                for kk in range(4):
                                    for j in range(2):
                                        ktile = 2 * kk + j
                                        nc.tensor.matmul(
                                            po[0:DH + 1, :],
                                            vp[ktile][:, h * 65:(h + 1) * 65],
                                            es_pair[hi][kk][:, j * 512:(j + 1) * 512],
                                            start=(kk == 0 and j == 0),
                                            stop=(kk == 3 and j == 1))
                                # custom-DVE ops require base partition 0,
                                # so stage the denominator row (psum row 64)
                                # into a partition-0 tile first.
                                den = scp.tile([1, 512], F32, tag="den",
                                               name="den", bufs=2)
                                nc.vector.tensor_copy(
                                    den[:], po[DH:DH + 1, :])
                                rc = scp.tile([1, 512], F32, tag="rc",
                                              name="rc", bufs=2)
                                nc.vector.reciprocal_approx_fast(
                                    rc[:], den[:])
                                bct = scp.tile([64, 512], F32,
                                               tag="bct", name="bct",
                                               bufs=2)
                                nc.gpsimd.partition_broadcast(
                                    bct[:], rc[:])
                                off = hi * 64
                                nc.vector.tensor_tensor(
                                    at[hp_][off:off + 64, :],
                                    po[0:DH, :], bct[:], OP.mult)

                        # out-projection, residual add into xt
                        def ev_o(m, ps):
                            tmp = scp.tile([P, 512], BF16, tag="tmp",
                                           name="tmp")
                            nc.vector.tensor_scalar_add(
                                tmp[:], ps, par[:, 24 + m:25 + m])
                            nc.vector.tensor_tensor(
                                xt[m][:, cs], xt[m][:, cs],
                                tmp[:], OP.add)
                        if debug and l == 0 and qc == 0:
                            for i in range(ET):
                                nc.sync.dma_start(
                                    dbg_q[P * i:P * (i + 1), :], qt[i][:])
                                nc.sync.dma_start(
                                    dbg_at[P * i:P * (i + 1), :], at[i][:])
                                nc.sync.dma_start(
                                    dbg_k[P * i:P * (i + 1), :], kt[i][:])
                            for t in range(KT):
                                nc.sync.dma_start(dbg_v[t], vp[t][:])
                        rhs_a = [at[k][:] for k in range(ET)]
                        for mg in range(2):
                            gemm_512(woT[l], 0, mg, rhs_a, ET, ev_o,
                                     gemm_ps4())

                # ---- Phase 3: LN1 + FFN + LN2 per q-chunk ----
                if skip_ffn:
                    continue
                with tc.tile_pool(name="psf", bufs=8, space="PSUM") as psf:
                    def ps_f():
                        return psf.tile([P, 512], F32, tag="ps", name="ps")

                    ln1 = [ln_stats(qc, psf, sc2) for qc in range(NQ)]
                    ln2 = [None] * NQ
                    for qc in range(NQ):
                        cs = slice(qc * 512, (qc + 1) * 512)
                        ln_apply(qc, ln1[qc][0], ln1[qc][1], 72, 80, par,
                                 sc2)
                        ht = [hpool.tile([P, 512], BF16, tag=f"h{i}",
                                         name=f"h{i}") for i in range(FT)]
                        rhs_x = [xt[k][:, cs] for k in range(ET)]

                        def ev_h(m, ps):
                            nc.scalar.activation(
                                ht[m][:], ps, AF.Relu,
                                bias=par[:, 32 + m:33 + m])
                        for mg in range(8):
                            gemm_512(w1T[l], 0, mg, rhs_x, ET, ev_h,
                                     [ps_f() for _ in range(4)])

                        def ev_f2(m, ps):
                            tmp = sc2.tile([P, 512], BF16, tag="tmp",
                                           name="tmp", bufs=2)
                            nc.vector.tensor_scalar_add(
                                tmp[:], ps, par[:, 64 + m:65 + m])
                            nc.vector.tensor_tensor(
                                xt[m][:, cs], xt[m][:, cs], tmp[:],
                                OP.add)
                        rhs_h = [ht[k][:] for k in range(FT)]
                        for mg in range(2):
                            gemm_512(w2T[l], 0, mg, rhs_h, FT, ev_f2,
                                     [ps_f() for _ in range(4)])
                        ln2[qc] = ln_stats(qc, psf, sc2)
                        if qc > 0:
                            mb, vb = ln2[qc - 1]
                            ln_apply(qc - 1, mb, vb, 88, 96, par, sc2)
                            if l == nlayers - 1:
                                ps = slice((qc - 1) * 512, qc * 512)
                                for i in range(ET):
                                    nc.sync.dma_start(
                                        outT[P * i:P * (i + 1), ps],
                                        xt[i][:, ps])
                    mb, vb = ln2[NQ - 1]
                    ln_apply(NQ - 1, mb, vb, 88, 96, par, sc2)
                    if l == nlayers - 1:
                        ps = slice((NQ - 1) * 512, NQ * 512)
                        for i in range(ET):
                            nc.sync.dma_start(outT[P * i:P * (i + 1), ps],
                                              xt[i][:, ps])

            if nlayers < L or skip_ffn:
                # debug configs: dump whatever state xt holds
                for i in range(ET):
                    nc.sync.dma_start(outT[P * i:P * (i + 1), :], xt[i][:])

    nc.compile()
    return nc


def _prep_inputs(inputs):
    """Host-side: transpose weights / pack params; returns per-core in_maps."""
    bf16 = ml_dtypes.bfloat16
    emb = np.asarray(inputs["embeddings"], dtype=np.float32)
    ipw = np.asarray(inputs["in_proj_w"], dtype=np.float32)   # [L, 3E, E]
    ipb = np.asarray(inputs["in_proj_b"], dtype=np.float32)   # [L, 3E]
    ow = np.asarray(inputs["out_w"], dtype=np.float32)        # [L, E, E]
    ob = np.asarray(inputs["out_b"], dtype=np.float32)        # [L, E]
    l1w = np.asarray(inputs["lin1_w"], dtype=np.float32)      # [L, DFF, E]
    l1b = np.asarray(inputs["lin1_b"], dtype=np.float32)      # [L, DFF]
    l2w = np.asarray(inputs["lin2_w"], dtype=np.float32)      # [L, E, DFF]
    l2b = np.asarray(inputs["lin2_b"], dtype=np.float32)      # [L, E]
    g1 = np.asarray(inputs["ln1_w"], dtype=np.float32)
    be1 = np.asarray(inputs["ln1_b"], dtype=np.float32)
    g2 = np.asarray(inputs["ln2_w"], dtype=np.float32)
    be2 = np.asarray(inputs["ln2_b"], dtype=np.float32)

    wqkvT = np.ascontiguousarray(ipw.transpose(0, 2, 1)).astype(bf16)
    woT = np.ascontiguousarray(ow.transpose(0, 2, 1)).astype(bf16)
    w1T = np.ascontiguousarray(l1w.transpose(0, 2, 1)).astype(bf16)
    w2T = np.ascontiguousarray(l2w.transpose(0, 2, 1)).astype(bf16)

    def cols(a, n):  # [L, n*128] -> [L, 128, n]
        return a.reshape(L, n, P).transpose(0, 2, 1)

    params = np.concatenate([
        cols(ipb[:, 0:E], 8), cols(ipb[:, E:2 * E], 8), cols(ipb[:, 2 * E:], 8),
        cols(ob, 8), cols(l1b, 32), cols(l2b, 8),
        cols(g1, 8), cols(be1, 8), cols(g2, 8), cols(be2, 8),
    ], axis=2)
    params = np.ascontiguousarray(params, dtype=np.float32)   # [L, 128, 104]
    bvrow = np.ascontiguousarray(ipb[:, 2 * E:3 * E])         # [L, E]

    shared = dict(wqkvT=wqkvT, woT=woT, w1T=w1T, w2T=w2T,
                  params=params, bvrow=bvrow,
                  onesb=np.ones((P, 16), bf16))
    in_maps = []
    for c in range(B):
        m = dict(shared)
        m["xT"] = np.ascontiguousarray(emb[c].T).astype(bf16)  # [E, S]
        in_maps.append(m)
    return in_maps


_NC_CACHE = {}


def _get_nc():
    if "nc" not in _NC_CACHE:
        _NC_CACHE["nc"] = build_encoder()
    return _NC_CACHE["nc"]


def run(inputs, trace=False, tmpdir=None):
    """Run on 8 NeuronCores; returns (output [8, S, E], BassKernelResults)."""
    in_maps = _prep_inputs(inputs)
    nc = _get_nc()
    res = bass_utils.run_bass_kernel_spmd(
        nc, in_maps, core_ids=list(range(B)), trace=trace, tmpdir=tmpdir)
    out = np.stack([np.ascontiguousarray(
        res.results[c]["outT"].astype(np.float32).T) for c in range(B)])
    return out, res


def kernel(**inputs):
    num_ctx = int(np.asarray(inputs["num_ctx"]))
    assert num_ctx == NC, f"kernel hardcodes num_ctx={NC}, got {num_ctx}"
    out, _ = run(inputs)
    return out


# revision 17
# speedup vs baseline: 1.0206x; 1.0206x over previous
"""Trainium2 Bass kernel for nn_Encoder_51900384804901.

6-layer post-norm TransformerEncoder (E=1024, NH=16, DFF=4096, relu FFN)
where every token attends only to the first num_ctx=1024 context tokens.

Sharding: data-parallel over batch. B=8 -> one batch element per NeuronCore,
no collectives. Each core runs the full 6-layer encoder on its [2048, 1024]
slice with activations resident in SBUF (feature-major x^T [E, T]) and
weights streamed from HBM.

All-bf16 datapath (PSUM accumulation stays fp32); the PE is kept dense so the
HAM clock gate stays at 2.4 GHz:
  - Softmax denominator reciprocal via reciprocal_approx_fast (single DVE op,
    ~5x faster than iterative divide) so the PSUM po tile frees quickly and
    the PE never stalls on attention-normalize chains. Custom-DVE ops only
    work at base partition 0, so the denominator row (psum row 64) is staged
    through a partition-0 tile first.
  - SBUF pools hoisted out of the layer loop; K/V/FFN weights stream through
    one shared pool so next-layer weight DMAs prefetch during the current
    layer's compute (removes the 22us layer-boundary PE gap).
  - LN rstd via a single Abs_reciprocal_sqrt activation: its act-table set
    also holds square/relu/identity, so the ACT table switches only twice per
    layer (exp set <-> abs_rsqrt set); using ln+exp instead thrashed the
    table ~16x per layer (~124us).
"""

import os
import numpy as np
import ml_dtypes

import concourse.bacc as bacc
import concourse.tile as tile
from concourse import mybir
from concourse import bass_utils

# Problem dims (hardcoded per contract)
L, E, NH, DFF = 6, 1024, 16, 4096
B, S, NC = 8, 2048, 1024
DH = E // NH  # 64
LN_EPS = 1e-5

F32 = mybir.dt.float32
BF16 = mybir.dt.bfloat16

P = 128          # partitions
NQ = S // 512    # 4 q-chunks of 512
NCC = NC // 512  # 2 ctx chunks of 512
ET = E // P      # 8 e-tiles
FT = DFF // P    # 32 f-tiles
KT = NC // P     # 8 ctx k-tiles


def build_encoder():
    nc = bacc.Bacc("TRN2", debug=False)

    xT = nc.dram_tensor("xT", [E, S], BF16, kind="ExternalInput").ap()
    wqkvT = nc.dram_tensor("wqkvT", [L, E, 3 * E], BF16, kind="ExternalInput").ap()
    woT = nc.dram_tensor("woT", [L, E, E], BF16, kind="ExternalInput").ap()
    w1T = nc.dram_tensor("w1T", [L, E, DFF], BF16, kind="ExternalInput").ap()
    w2T = nc.dram_tensor("w2T", [L, DFF, E], BF16, kind="ExternalInput").ap()
    # params[l]: [128, 104] per-partition param columns:
    # 0-7 bq | 8-15 bk | 16-23 bv | 24-31 bo | 32-63 b1 | 64-71 b2
    # 72-79 g1 | 80-87 be1 | 88-95 g2 | 96-103 be2   (col m <-> e-tile m)
    params = nc.dram_tensor("params", [L, P, 104], F32, kind="ExternalInput").ap()
    bvrow = nc.dram_tensor("bvrow", [L, E], F32, kind="ExternalInput").ap()
    onesb = nc.dram_tensor("onesb", [P, 16], BF16, kind="ExternalInput").ap()
    outT = nc.dram_tensor("outT", [E, S], BF16, kind="ExternalOutput").ap()
    debug = os.environ.get("ENC_DEBUG", "") == "1"
    if debug:
        dbg_q = nc.dram_tensor("dbg_q", [E, 512], BF16, kind="ExternalOutput").ap()
        dbg_k = nc.dram_tensor("dbg_k", [E, NC], BF16, kind="ExternalOutput").ap()
        dbg_v = nc.dram_tensor("dbg_v", [KT, P, NH * (DH + 1)], BF16, kind="ExternalOutput").ap()
        dbg_es = nc.dram_tensor("dbg_es", [4, P, 1024], BF16, kind="ExternalOutput").ap()
        dbg_at = nc.dram_tensor("dbg_at", [E, 512], BF16, kind="ExternalOutput").ap()

    AF = mybir.ActivationFunctionType
    OP = mybir.AluOpType

    with tile.TileContext(nc) as tc:
        with tc.tile_pool(name="persist", bufs=1) as pp, \
             tc.tile_pool(name="wstream", bufs=6) as wsp, \
             tc.tile_pool(name="wv", bufs=1) as wvp, \
             tc.tile_pool(name="kv", bufs=1) as kvp, \
             tc.tile_pool(name="att", bufs=1) as att, \
             tc.tile_pool(name="es", bufs=12) as esp, \
             tc.tile_pool(name="sc", bufs=3) as scp, \
             tc.tile_pool(name="hp", bufs=1) as hpool, \
             tc.tile_pool(name="sc2", bufs=3) as sc2, \
             tc.tile_pool(name="parp", bufs=2) as parp:
            # Residual stream x^T, resident for the whole kernel (bf16)
            xt = [pp.tile([P, S], BF16, tag=f"x{i}", name=f"x{i}")
                  for i in range(ET)]
            # ctx columns first (512-col chunks) so layer-0 K/V projection
            # starts as soon as the first 1MB lands
            for c0 in range(0, NC, 512):
                for i in range(ET):
                    nc.sync.dma_start(xt[i][:, c0:c0 + 512],
                                      xT[P * i:P * (i + 1), c0:c0 + 512])
            for i in range(ET):
                nc.sync.dma_start(xt[i][:, NC:S], xT[P * i:P * (i + 1), NC:S])
            ones = pp.tile([P, 1], BF16, name="ones")
            nc.sync.dma_start(ones[:], onesb[:, 0:1])
            eps_t = pp.tile([P, 1], F32, tag="eps", name="eps")
            nc.vector.memset(eps_t[:], LN_EPS)

            def gemm_512(wdram, col_off, mg, rhs_tiles, nk, evict, ps4):
                """out[m] = sum_k W[k, col_off+mg*512+mi*128 ...].T @ rhs[k]
                for the 4 m-subtiles of one 512-wide weight column group.
                ps4: list of 4 one-bank psum APs."""
                c0 = col_off + mg * 512
                for k in range(nk):
                    w = wsp.tile([P, 512], BF16, tag="w", name="w")
                    nc.sync.dma_start(
                        w[:], wdram[P * k:P * (k + 1), c0:c0 + 512])
                    for mi in range(4):
                        nc.tensor.matmul(
                            ps4[mi],
                            w[:, P * mi:P * (mi + 1)],
                            rhs_tiles[k],
                            start=(k == 0), stop=(k == nk - 1))
                for mi in range(4):
                    evict(mg * 4 + mi, ps4[mi])

            def ln_stats(qc, ps_tile, ln_pool):
                """Partition-axis sums + broadcast-form mean / rstd tiles.
                Returns (mean_b, rstd_b) [128,512] f32 SBUF tiles."""
                cs = slice(qc * 512, (qc + 1) * 512)
                s1 = ps_tile()
                for k in range(ET):
                    nc.tensor.matmul(s1[0:1, :], ones[:], xt[k][:, cs],
                                     start=(k == 0), stop=(k == ET - 1))
                s2 = ps_tile()
                for k in range(ET):
                    sq = ln_pool.tile([P, 512], BF16, tag="sq", name="sq",
                                      bufs=2)
                    nc.vector.tensor_tensor(sq[:], xt[k][:, cs],
                                            xt[k][:, cs], OP.mult)
                    nc.tensor.matmul(s2[0:1, :], ones[:], sq[:],
                                     start=(k == 0), stop=(k == ET - 1))
                m1 = ln_pool.tile([1, 512], F32, tag="m1", name="m1", bufs=1)
                nc.vector.tensor_scalar_mul(m1[:], s1[0:1, :], 1.0 / E)
                m2 = ln_pool.tile([1, 512], F32, tag="m2", name="m2", bufs=1)
                nc.vector.tensor_scalar_mul(m2[:], s2[0:1, :], 1.0 / E)
                mb = ln_pool.tile([P, 512], F32, tag="mb", name="mb", bufs=2)
                nc.gpsimd.partition_broadcast(mb[:], m1[:])
                vb = ln_pool.tile([P, 512], F32, tag="vb", name="vb", bufs=2)
                nc.gpsimd.partition_broadcast(vb[:], m2[:])
                # var = E[x^2] - mean^2 ;  rstd = 1/sqrt(var+eps).
                # Single ACT op; its table set also holds square/relu/identity
                # so the act table switches only twice per layer.
                msq = ln_pool.tile([P, 512], F32, tag="msq", name="msq",
                                   bufs=1)
                nc.vector.tensor_tensor(msq[:], mb[:], mb[:], OP.mult)
                nc.vector.tensor_tensor(vb[:], vb[:], msq[:], OP.subtract)
                nc.scalar.activation(vb[:], vb[:], AF.Abs_reciprocal_sqrt,
                                     bias=eps_t[:])
                return mb, vb

            def ln_apply(qc, mb, vb, g_col, b_col, par, ln_pool):
                cs = slice(qc * 512, (qc + 1) * 512)
                for k in range(ET):
                    t1 = ln_pool.tile([P, 512], BF16, tag="t1", name="t1",
                                      bufs=2)
                    nc.vector.tensor_tensor(t1[:], xt[k][:, cs], mb[:],
                                            OP.subtract)
                    nc.vector.tensor_tensor(t1[:], t1[:], vb[:], OP.mult)
                    nc.vector.tensor_scalar(
                        xt[k][:, cs], t1[:],
                        par[:, g_col + k:g_col + k + 1],
                        par[:, b_col + k:b_col + k + 1],
                        OP.mult, OP.add)

            nlayers = int(os.environ.get("ENC_NLAYERS", L))
            skip_ffn = os.environ.get("ENC_SKIP_FFN", "") == "1"
            for l in range(nlayers):
                par = parp.tile([P, 104], F32, tag="par", name="par")
                nc.sync.dma_start(par[:], params[l])

                # ---- Phase 1+2: K/V then attention over all q-chunks ----
                kt = [kvp.tile([P, NC], BF16, tag=f"k{i}", name=f"k{i}")
                      for i in range(ET)]
                vp = [kvp.tile([P, NH * (DH + 1)], BF16, tag=f"v{i}",
                               name=f"v{i}") for i in range(KT)]
                with tc.tile_pool(name="psa", bufs=1, space="PSUM") as psa:
                    def s2_tile():
                        return psa.tile([P, 1024], F32, tag="s2",
                                        name="s2", bufs=3)

                    def po_tile():
                        return psa.tile([P, 512], F32, tag="po",
                                        name="po", bufs=2)

                    def gemm_ps4():
                        a, b = s2_tile(), s2_tile()
                        return [a[:, 0:512], a[:, 512:1024],
                                b[:, 0:512], b[:, 512:1024]]

                    bvb = scp.tile([P, E], F32, tag="bvb", name="bvb",
                                   bufs=1)
                    bvr = scp.tile([1, E], F32, tag="bvr", name="bvr",
                                   bufs=1)
                    nc.sync.dma_start(bvr[:], bvrow[l][None, :])
                    nc.gpsimd.partition_broadcast(bvb[:], bvr[:])

                    # K projection (feature-major out), streamed weights
                    for cc in range(NCC):
                        ccs = slice(cc * 512, (cc + 1) * 512)
                        rhs_ctx = [xt[k][:, ccs] for k in range(ET)]

                        def ev_k(m, ps, _ccs=ccs):
                            nc.vector.tensor_scalar_add(
                                kt[m][:, _ccs], ps, par[:, 8 + m:9 + m])
                        for mg in range(2):
                            gemm_512(wqkvT[l], E, mg, rhs_ctx, ET, ev_k,
                                     gemm_ps4())

                    # V projection (token-major out), x_ctx as lhsT
                    wv = [wvp.tile([P, E], BF16, tag=f"wv{k}",
                                   name=f"wv{k}") for k in range(ET)]
                    for k in range(ET):
                        nc.sync.dma_start(
                            wv[k][:],
                            wqkvT[l, P * k:P * (k + 1), 2 * E:3 * E])
                    for t in range(KT):
                        for ch in range(2):
                            ps = po_tile()
                            for k in range(ET):
                                nc.tensor.matmul(
                                    ps[:],
                                    xt[k][:, P * t:P * (t + 1)],
                                    wv[k][:, ch * 512:(ch + 1) * 512],
                                    start=(k == 0),
                                    stop=(k == ET - 1))
                            for hh in range(8):
                                h = ch * 8 + hh
                                nc.vector.tensor_tensor(
                                    vp[t][:, h * 65:h * 65 + 64],
                                    ps[:, hh * 64:(hh + 1) * 64],
                                    bvb[:, h * 64:(h + 1) * 64],
                                    OP.add)
                        ones_cols = vp[t].rearrange(
                            "p (h c) -> p h c", c=DH + 1)[:, :, 64:65]
                        nc.sync.dma_start(ones_cols, onesb[:, :, None])

                    # Attention + out-projection per q-chunk
                    for qc in range(NQ):
                        cs = slice(qc * 512, (qc + 1) * 512)
                        qt = [att.tile([P, 512], BF16, tag=f"q{i}",
                                       name=f"q{i}")
                              for i in range(ET)]
                        rhs_x = [xt[k][:, cs] for k in range(ET)]

                        def ev_q(m, ps):
                            nc.vector.tensor_scalar_add(
                                qt[m][:], ps, par[:, m:m + 1])
                        for mg in range(2):
                            gemm_512(wqkvT[l], 0, mg, rhs_x, ET, ev_q,
                                     gemm_ps4())

                        # attention per head pair (row-tiled quadrants)
                        at = [att.tile([P, 512], BF16, tag=f"a{i}",
                                       name=f"a{i}")
                              for i in range(ET)]
                        for hp_ in range(NH // 2):
                            h0, h1 = 2 * hp_, 2 * hp_ + 1
                            # one score tile per ktile: head hp row-group
                            # pairs write cols 0:512 (h0) / 512:1024 (h1) of
                            # the same 2-bank tile, so each QK step needs
                            # only ONE free score tile (smoother rotation
                            # when FFN gemms hold the others).
                            es_list = []  # 8 tiles, one per ktile
                            for ktile in range(KT):
                                ps1 = s2_tile()
                                for hi, off in ((0, 0), (1, 64)):
                                    nc.tensor.matmul(
                                        ps1[:, hi * 512:(hi + 1) * 512],
                                        kt[hp_][off:off + 64,
                                                P * ktile:P * (ktile + 1)],
                                        qt[hp_][off:off + 64, :],
                                        start=True, stop=True)
                                es = esp.tile([P, 1024], BF16,
                                              tag="es", name="es")
                                nc.scalar.activation(
                                    es[:], ps1[:], AF.Exp,
                                    scale=float(1.0 / np.sqrt(DH)))
                                es_list.append(es)
                            for hi, h in ((0, h0), (1, h1)):
                                po = po_tile()
                                for ktile in range(KT):
                                    nc.tensor.matmul(
                                        po[0:DH + 1, :],
                                        vp[ktile][:, h * 65:(h + 1) * 65],
                                        es_list[ktile][:, hi * 512:(hi + 1) * 512],
                                        start=(ktile == 0),
                                        stop=(ktile == KT - 1))
                                # custom-DVE ops require base partition 0,
                                # so stage the denominator row (psum row 64)
                                # into a partition-0 tile first.
                                den = scp.tile([1, 512], F32, tag="den",
                                               name="den", bufs=2)
                                nc.vector.tensor_copy(
                                    den[:], po[DH:DH + 1, :])
                                rc = scp.tile([1, 512], F32, tag="rc",
                                              name="rc", bufs=2)
                                nc.vector.reciprocal_approx_fast(
                                    rc[:], den[:])
                                bct = scp.tile([64, 512], F32,
                                               tag="bct", name="bct",
                                               bufs=3)
                                nc.gpsimd.partition_broadcast(
                                    bct[:], rc[:])
                                off = hi * 64
                                nc.vector.tensor_tensor(
                                    at[hp_][off:off + 64, :],
                                    po[0:DH, :], bct[:], OP.mult)

                        # out-projection, residual add into xt
                        def ev_o(m, ps):
                            tmp = scp.tile([P, 512], BF16, tag="tmp",
                                           name="tmp")
                            nc.vector.tensor_scalar_add(
                                tmp[:], ps, par[:, 24 + m:25 + m])
                            nc.vector.tensor_tensor(
                                xt[m][:, cs], xt[m][:, cs],
                                tmp[:], OP.add)
                        if debug and l == 0 and qc == 0:
                            for i in range(ET):
                                nc.sync.dma_start(
                                    dbg_q[P * i:P * (i + 1), :], qt[i][:])
                                nc.sync.dma_start(
                                    dbg_at[P * i:P * (i + 1), :], at[i][:])
                                nc.sync.dma_start(
                                    dbg_k[P * i:P * (i + 1), :], kt[i][:])
                            for t in range(KT):
                                nc.sync.dma_start(dbg_v[t], vp[t][:])
                        rhs_a = [at[k][:] for k in range(ET)]
                        for mg in range(2):
                            gemm_512(woT[l], 0, mg, rhs_a, ET, ev_o,
                                     gemm_ps4())

                        # ---- LN1 + FFN + LN2 for this q-chunk, inside the
                        # same psum scope: FFN gemms rotate through the s2
                        # tag and LN sums through po, so the scheduler can
                        # fill exp-stall PE holes of attention(qc+1) with
                        # FFN(qc) matmuls and vice versa. ----
                        if skip_ffn:
                            continue
                        mb1, vb1 = ln_stats(qc, po_tile, sc2)
                        ln_apply(qc, mb1, vb1, 72, 80, par, sc2)
                        ht = [hpool.tile([P, 512], BF16, tag=f"h{i}",
                                         name=f"h{i}") for i in range(FT)]

                        def ev_h(m, ps):
                            nc.scalar.activation(
                                ht[m][:], ps, AF.Relu,
                                bias=par[:, 32 + m:33 + m])
                        for mg in range(8):
                            gemm_512(w1T[l], 0, mg, rhs_x, ET, ev_h,
                                     gemm_ps4())

                        def ev_f2(m, ps):
                            tmp = sc2.tile([P, 512], BF16, tag="tmp",
                                           name="tmp", bufs=2)
                            nc.vector.tensor_scalar_add(
                                tmp[:], ps, par[:, 64 + m:65 + m])
                            nc.vector.tensor_tensor(
                                xt[m][:, cs], xt[m][:, cs], tmp[:],
                                OP.add)
                        rhs_h = [ht[k][:] for k in range(FT)]
                        for mg in range(2):
                            gemm_512(w2T[l], 0, mg, rhs_h, FT, ev_f2,
                                     gemm_ps4())
                        mb2, vb2 = ln_stats(qc, po_tile, sc2)
                        ln_apply(qc, mb2, vb2, 88, 96, par, sc2)
                        if l == nlayers - 1:
                            for i in range(ET):
                                nc.sync.dma_start(
                                    outT[P * i:P * (i + 1), cs],
                                    xt[i][:, cs])

            if nlayers < L or skip_ffn:
                # debug configs: dump whatever state xt holds
                for i in range(ET):
                    nc.sync.dma_start(outT[P * i:P * (i + 1), :], xt[i][:])

    nc.compile()
    return nc


def _prep_inputs(inputs):
    """Host-side: transpose weights / pack params; returns per-core in_maps."""
    bf16 = ml_dtypes.bfloat16
    emb = np.asarray(inputs["embeddings"], dtype=np.float32)
    ipw = np.asarray(inputs["in_proj_w"], dtype=np.float32)   # [L, 3E, E]
    ipb = np.asarray(inputs["in_proj_b"], dtype=np.float32)   # [L, 3E]
    ow = np.asarray(inputs["out_w"], dtype=np.float32)        # [L, E, E]
    ob = np.asarray(inputs["out_b"], dtype=np.float32)        # [L, E]
    l1w = np.asarray(inputs["lin1_w"], dtype=np.float32)      # [L, DFF, E]
    l1b = np.asarray(inputs["lin1_b"], dtype=np.float32)      # [L, DFF]
    l2w = np.asarray(inputs["lin2_w"], dtype=np.float32)      # [L, E, DFF]
    l2b = np.asarray(inputs["lin2_b"], dtype=np.float32)      # [L, E]
    g1 = np.asarray(inputs["ln1_w"], dtype=np.float32)
    be1 = np.asarray(inputs["ln1_b"], dtype=np.float32)
    g2 = np.asarray(inputs["ln2_w"], dtype=np.float32)
    be2 = np.asarray(inputs["ln2_b"], dtype=np.float32)

    wqkvT = np.ascontiguousarray(ipw.transpose(0, 2, 1)).astype(bf16)
    woT = np.ascontiguousarray(ow.transpose(0, 2, 1)).astype(bf16)
    w1T = np.ascontiguousarray(l1w.transpose(0, 2, 1)).astype(bf16)
    w2T = np.ascontiguousarray(l2w.transpose(0, 2, 1)).astype(bf16)

    def cols(a, n):  # [L, n*128] -> [L, 128, n]
        return a.reshape(L, n, P).transpose(0, 2, 1)

    params = np.concatenate([
        cols(ipb[:, 0:E], 8), cols(ipb[:, E:2 * E], 8), cols(ipb[:, 2 * E:], 8),
        cols(ob, 8), cols(l1b, 32), cols(l2b, 8),
        cols(g1, 8), cols(be1, 8), cols(g2, 8), cols(be2, 8),
    ], axis=2)
    params = np.ascontiguousarray(params, dtype=np.float32)   # [L, 128, 104]
    bvrow = np.ascontiguousarray(ipb[:, 2 * E:3 * E])         # [L, E]

    shared = dict(wqkvT=wqkvT, woT=woT, w1T=w1T, w2T=w2T,
                  params=params, bvrow=bvrow,
                  onesb=np.ones((P, 16), bf16))
    in_maps = []
    for c in range(B):
        m = dict(shared)
        m["xT"] = np.ascontiguousarray(emb[c].T).astype(bf16)  # [E, S]
        in_maps.append(m)
    return in_maps


_NC_CACHE = {}


def _get_nc():
    if "nc" not in _NC_CACHE:
        _NC_CACHE["nc"] = build_encoder()
    return _NC_CACHE["nc"]


def run(inputs, trace=False, tmpdir=None):
    """Run on 8 NeuronCores; returns (output [8, S, E], BassKernelResults)."""
    in_maps = _prep_inputs(inputs)
    nc = _get_nc()
    res = bass_utils.run_bass_kernel_spmd(
        nc, in_maps, core_ids=list(range(B)), trace=trace, tmpdir=tmpdir)
    out = np.stack([np.ascontiguousarray(
        res.results[c]["outT"].astype(np.float32).T) for c in range(B)])
    return out, res


def kernel(**inputs):
    num_ctx = int(np.asarray(inputs["num_ctx"]))
    assert num_ctx == NC, f"kernel hardcodes num_ctx={NC}, got {num_ctx}"
    out, _ = run(inputs)
    return out
